# revision 47
# baseline (speedup 1.0000x reference)
"""Causal single-head attention (B=16, S=2048, D=1024, HD=64) on 8 TRN2 cores.

Data-parallel: 2 batches per core. The host pre-transposes X to X^T layout
[BPC, 8, 128, S] and casts to bf16, so the contraction dim (D) streams from
HBM straight onto partitions — no on-device transposes or PSUM->SBUF copies
of X, and half the DMA bytes.

Per batch, streamed over 4 seq-tiles of 512:
  - two packed projection passes: stationary [Wk|Wq] and [Wq|Wv] (bf16) over
    X^T tiles; DVE applies biases while casting PSUM->bf16 SBUF,
  - S^T = K @ Q^T per (kseq-block, qtile) with causal block skipping and
    diagonal narrowing; the causal mask is a -1000 accumulate-matmul on PE
    (so exp(masked) == 0 and no vector mask pass is needed); one wide exp on
    ScalarE per pair of blocks (2-bank PSUM tile) with the 1/sqrt(64) scale
    folded in, bf16 out,
  - O^T accumulation with stationary [V | ones*64] (bf16): the ones columns
    make the same matmul emit the softmax denominators pre-broadcast across
    rows 64:128,
  - normalization in transposed layout: sums -> SBUF, fast-approx reciprocal
    and multiply on DVE. Output is written transposed [64, S]; the host
    untransposes.

The emission is software-pipelined: while unit i's attention (paced by the
ScalarE exp stream) is emitted, the next units' loads/projections are
interleaved between attention blocks to keep the PE dense (HAM warm).
"""
import numpy as np
import ml_dtypes

import concourse.bacc as bacc
import concourse.mybir as mybir
import concourse.tile as tile
from concourse import bass_utils

B, S, D, HD = 16, 2048, 1024, 64
N_CORES = 8
BPC = B // N_CORES          # batches per core
ST = 512                    # seq tile (qtile) size
NST = S // ST               # 4 seq tiles per batch
NDB = D // 128              # 8 d-blocks
NKB = S // 128              # 16 kseq blocks per batch

f32 = mybir.dt.float32
bf16 = mybir.dt.bfloat16
np_bf16 = ml_dtypes.bfloat16

C16 = 128 + 128 + 1024 + 1024         # ident | maskT | wkq | wqv
C32 = 2                               # bias_kq | bias_qv

_cache = {}


def _build():
    nc = bacc.Bacc("TRN2", target_bir_lowering=False, debug=False,
                   num_devices=N_CORES)

    xt_d = nc.dram_tensor("xt", [BPC, NDB, 128, S], bf16, kind="ExternalInput")
    c16_d = nc.dram_tensor("c16", [128, C16], bf16, kind="ExternalInput")
    c32_d = nc.dram_tensor("c32", [128, C32], f32, kind="ExternalInput")
    out_t = nc.dram_tensor("out_t", [BPC, HD, S], f32, kind="ExternalOutput")

    with tile.TileContext(nc) as tc:
        with (
            tc.tile_pool(name="consts", bufs=1) as consts,
            tc.tile_pool(name="big", bufs=3) as big,
            tc.tile_pool(name="perbatch", bufs=1) as perbatch,
            tc.tile_pool(name="work", bufs=3) as work,
            tc.tile_pool(name="pp", bufs=1, space="PSUM") as pp,
            tc.tile_pool(name="ps_s", bufs=2, space="PSUM") as ps_s,
            tc.tile_pool(name="ps_o", bufs=2, space="PSUM") as ps_o,
        ):
            cst16 = consts.tile([128, C16], bf16)
            nc.sync.dma_start(out=cst16[:, 0:256], in_=c16_d.ap()[:, 0:256])
            nc.scalar.dma_start(out=cst16[:, 256:], in_=c16_d.ap()[:, 256:])
            cst32 = consts.tile([128, C32], f32)
            nc.scalar.dma_start(out=cst32, in_=c32_d.ap())

            ident = cst16[:, 0:128]
            maskT = cst16[:, 128:256]
            wkq = cst16[:, 256:1280].rearrange("p (db m) -> p db m", db=NDB)
            wqv = cst16[:, 1280:2304].rearrange("p (db m) -> p db m", db=NDB)
            bias_kq = cst32[:, 0:1]
            bias_qv = cst32[:, 1:2]

            kq_sbs, vn_sbs = [], []
            for b in range(BPC):
                # K^T per seqtile (rows 0:64; rows 64:128 hold an unused Q^T)
                kq_sb = perbatch.tile([128, NST, ST], bf16, name=f"kq_sb{b}")
                # V natural padded with 64 ones columns: the AV matmul then
                # emits O^T in rows 0:64 and the softmax denominators
                # replicated across rows 64:128 (pre-broadcast, for free)
                vn_sb = perbatch.tile([128, NKB, 128], bf16, name=f"vn_sb{b}")
                nc.gpsimd.memset(vn_sb[:, :, 64:128], 1.0)
                kq_sbs.append(kq_sb)
                vn_sbs.append(vn_sb)

            # warm the PE clock while the first X^T tiles are in flight:
            # one tile, back-to-back matmuls (PE is in-order; no cross-
            # engine deps beyond the small leading consts chunk)
            warm_ps = pp.tile([128, ST], f32, tag="p1", name="p1")
            for _ in range(24):
                nc.tensor.matmul(warm_ps[:, 0:128],
                                 cst16[:, 0:128], cst16[:, 0:128],
                                 start=True, stop=True)

            units = [(b, st) for b in range(BPC) for st in range(NST)]

            def filler_gen(b, st):
                """Emits unit (b, st)'s X^T loads, projections and V-natural
                prep incrementally (9 yield points)."""
                kq_sb, vn_sb = kq_sbs[b], vn_sbs[b]
                xt = big.tile([128, NDB, ST], bf16, tag="xt", name="xt")
                for db in range(NDB):
                    nc.sync.dma_start(
                        out=xt[:, db, :],
                        in_=xt_d.ap()[b, db, :, ST * st:ST * (st + 1)])
                p1 = pp.tile([128, ST], f32, tag="p1", name="p1")
                p2 = pp.tile([128, ST], f32, tag="p2", name="p2")
                for db in range(NDB):
                    nc.tensor.matmul(p1, wkq[:, db, :], xt[:, db, :],
                                     start=(db == 0), stop=(db == NDB - 1))
                    nc.tensor.matmul(p2, wqv[:, db, :], xt[:, db, :],
                                     start=(db == 0), stop=(db == NDB - 1))
                    yield None
                nc.vector.tensor_scalar_add(out=kq_sb[:, st, :], in0=p1,
                                            scalar1=bias_kq)
                qv_sb = work.tile([128, ST], bf16, tag="qv", bufs=6,
                                  name="qv_sb")
                nc.vector.tensor_scalar_add(out=qv_sb, in0=p2,
                                            scalar1=bias_qv)
                vn_ps = ps_s.tile([128, 4, 64], bf16, tag="s_ps", name="vn_ps")
                for c in range(4):
                    nc.tensor.transpose(
                        vn_ps[:, c, :],
                        qv_sb[64:128, 128 * c:128 * (c + 1)],
                        ident[64:128, 64:128])
                nc.vector.tensor_copy(
                    out=vn_sb[:, 4 * st:4 * st + 4, 0:64], in_=vn_ps)
                yield qv_sb

            def emit_finalize(b, st, o_ps):
                sums_sb = work.tile([HD, ST], f32, tag="sm", bufs=2,
                                    name="sums_sb")
                nc.vector.tensor_copy(out=sums_sb, in_=o_ps[64:128, :])
                recip_bc = work.tile([HD, ST], f32, tag="rc", bufs=2,
                                     name="recip_bc")
                nc.vector.reciprocal_approx_fast(out=recip_bc, in_=sums_sb)
                o_out = work.tile([HD, ST], f32, tag="oo", bufs=2,
                                  name="o_out")
                nc.vector.tensor_mul(out=o_out, in0=o_ps[0:HD, :],
                                     in1=recip_bc)
                nc.sync.dma_start(
                    out=out_t.ap()[b, :, ST * st:ST * (st + 1)], in_=o_out)

            def fill_chain():
                for i, (b, st) in enumerate(units):
                    g = filler_gen(b, st)
                    while True:
                        try:
                            r = next(g)
                        except StopIteration:
                            break
                        if r is not None:
                            yield ("unit", i, r)
                        else:
                            yield ("step", i)

            chain = fill_chain()
            qv_ready = {}

            def pull_until_unit(i):
                for kind, *rest in chain:
                    if kind == "unit":
                        qv_ready[rest[0]] = rest[1]
                        if rest[0] >= i:
                            return

            def pull_steps(n):
                got = 0
                while got < n:
                    try:
                        kind, *rest = next(chain)
                    except StopIteration:
                        exhausted[0] = True
                        return
                    if kind == "unit":
                        qv_ready[rest[0]] = rest[1]
                    else:
                        got += 1

            exhausted = [False]
            # pend entries carry their unit's refs so AV emission can lag
            # across unit boundaries: the PE is in-order, so draining a
            # unit's last AVs (which wait on its last exps) before emitting
            # the next unit's scores head-of-line-blocks the queue
            pends = []

            def pop_pend():
                (p_ps, p_vn, pe_pair, pwidths, is_last, pb, pst) = \
                    pends.pop(0)
                last = len(pwidths) - 1
                for n, (pw, poff, pkb) in enumerate(pwidths):
                    nc.tensor.matmul(
                        p_ps[:, poff:poff + pw],
                        p_vn[:, pkb, :],
                        pe_pair[:, n, 0:pw],
                        start=(pkb == 0), stop=(is_last and n == last))
                if is_last:
                    emit_finalize(pb, pst, p_ps)

            for i, (b, st) in enumerate(units):
                if i not in qv_ready:
                    pull_until_unit(i)
                qv_sb = qv_ready.pop(i)
                kq_sb, vn_sb = kq_sbs[b], vn_sbs[b]
                o_ps = ps_o.tile([128, ST], f32, name="o_ps")
                n_att = 4 * st + 4

                for pk in range(n_att // 2):
                    # two kseq blocks share one 2-bank PSUM tile so a single
                    # wide exp amortizes the ScalarE fixed overhead
                    s_pair = ps_s.tile([128, 2, ST], f32, tag="s_ps",
                                       name="s_pair")
                    e_pair = work.tile([128, 2, ST], bf16, tag="e", bufs=8,
                                       name="e_pair")
                    widths = []
                    for half in range(2):
                        kb = 2 * pk + half
                        j = kb - 4 * st
                        if j < 0:
                            w, qoff = ST, 0
                        else:
                            w, qoff = ST - 128 * j, 128 * j
                        nc.tensor.matmul(
                            s_pair[:, half, 0:w],
                            kq_sb[0:64, kb // 4,
                                  128 * (kb % 4):128 * (kb % 4) + 128],
                            qv_sb[0:64, qoff:qoff + w],
                            start=True, stop=(j < 0))
                        if j >= 0:
                            # causal mask: add -1000 above the diagonal so
                            # exp underflows to zero
                            nc.tensor.matmul(
                                s_pair[:, half, 0:128], maskT, ident,
                                start=False, stop=True)
                        widths.append((w, qoff, kb))
                    # exp both halves in one instruction; the tile is
                    # contiguous across its two banks, so cols [0, 512+w2)
                    # cover half0 fully (w1<512 leaves unread garbage cols)
                    wtot = ST + widths[1][0]
                    nc.scalar.activation(
                        out=e_pair.rearrange("p a b -> p (a b)")[:, 0:wtot],
                        in_=s_pair.rearrange("p a b -> p (a b)")[:, 0:wtot],
                        func=mybir.ActivationFunctionType.Exp,
                        scale=float(HD) ** -0.5)
                    if len(pends) >= 6:
                        pop_pend()
                    pends.append((o_ps, vn_sb, e_pair, widths,
                                  pk == n_att // 2 - 1, b, st))
                    pull_steps(2)
                    if exhausted[0]:
                        # PE heater: real matmuls into the idle projection
                        # bank keep HAM at full clock through the exp-paced
                        # tail
                        heat_ps = pp.tile([128, ST], f32, tag="p1",
                                          name="p1")
                        for _ in range(2):
                            nc.tensor.matmul(
                                heat_ps[:, 0:128],
                                cst16[:, 0:128], cst16[:, 0:128],
                                start=True, stop=True)
            while pends:
                pop_pend()

    nc.compile()
    return nc


def _pack_consts(wq, wk, wv, bq, bk, bv):
    def packed_pair(wa, wb):
        pa = wa.reshape(NDB, 128, HD).transpose(1, 0, 2)   # [128, db, 64]
        pb = wb.reshape(NDB, 128, HD).transpose(1, 0, 2)
        return np.concatenate([pa, pb], axis=2).reshape(128, NDB * 128)

    c16 = np.zeros((128, C16), dtype=np.float32)
    c16[:, 0:128] = np.eye(128, dtype=np.float32)
    # maskT[q, k] = -1000 where k > q: PE-accumulated onto the diagonal
    # scores block so exp maps masked entries to zero
    c16[:, 128:256] = np.where(
        np.arange(128)[None, :] > np.arange(128)[:, None], -1000.0, 0.0)
    c16[:, 256:1280] = packed_pair(wk, wq)
    c16[:, 1280:2304] = packed_pair(wq, wv)

    c32 = np.zeros((128, C32), dtype=np.float32)
    c32[:, 0] = np.concatenate([bk, bq])
    c32[:, 1] = np.concatenate([bq, bv])
    return np.ascontiguousarray(c16.astype(np_bf16)), np.ascontiguousarray(c32)


def kernel(x, Wq, bq, Wk, bk, Wv, bv):
    if "nc" not in _cache:
        _cache["nc"] = _build()
    nc = _cache["nc"]

    # host-side layout prep: X^T [B, db, 128, S] in bf16
    x = np.asarray(x, dtype=np.float32)
    xt = np.ascontiguousarray(
        x.astype(np_bf16).reshape(B, S, NDB, 128).transpose(0, 2, 3, 1))
    c16, c32 = _pack_consts(np.asarray(Wq, np.float32),
                            np.asarray(Wk, np.float32),
                            np.asarray(Wv, np.float32),
                            np.asarray(bq, np.float32),
                            np.asarray(bk, np.float32),
                            np.asarray(bv, np.float32))

    in_maps = []
    for c in range(N_CORES):
        in_maps.append({
            "xt": xt[c * BPC:(c + 1) * BPC],
            "c16": c16,
            "c32": c32,
        })

    res = bass_utils.run_bass_kernel_spmd(nc, in_maps,
                                          core_ids=list(range(N_CORES)),
                                          **_cache.get("run_kwargs", {}))
    _cache["last_result"] = res
    # out_t is [BPC, HD, S] per core; untranspose and stack on host
    return np.ascontiguousarray(np.concatenate(
        [res.results[c]["out_t"].transpose(0, 2, 1) for c in range(N_CORES)],
        axis=0))
